# revision 36
# baseline (speedup 1.0000x reference)
"""BallQueryAttention TRN2 kernel.

Math: reference computes softmax over a binary ball mask (d2 <= R^2), then
mask-softmax @ x.  exp of a 0/1 mask takes only values {1, e}, so

  out[i] = (S + (e-1) * sum_{j in ball(i)} x_j) / (N + (e-1) * cnt_i)

with S = colsum([x|1]).  Sharding: rows (i) across 8 cores, x replicated.

Per core (row shard of 1024):
  - Gram half-tiles Gh[j(128 part), i(512 free)] via a SINGLE fp16 K=65
    matmul: stationary [x_j; 1] = XT column slice (XT is the DMA-transpose
    of the fp16 staging tile XH, whose column 64 holds ones), moving
    [x_i; -0.5*sq_i] (RM).  fp16 quantization flips ~2 mask bits/row out
    of ~3200 in-ball -> ~5e-3 L2, well inside tolerance.
  - mask compare split across Vector (is_ge -> {0,2}, i-cols 0:512) and
    Scalar (Sign -> {-1,1}, i-cols 512:1024) engines, writing fp16 masks.
  - accumulating [x|1]^T @ mask matmuls -> OUT2 [65, 1024] in PSUM, with
    the stationary [x_j|1] read as a strided 65-column view of XH (no
    separate XW tile).  The column-sum S rides 16 small matmuls against
    XH views:
      numer/denom cols 0:512    = S + K1*OUT2
      numer/denom cols 512:1024 = (1+K1)*S + K1*OUT2,   K1 = (e-1)/2
  - PE transpose + reciprocal + per-partition scale for the final divide.

Scheduling: engine queues are strict FIFO, so any preamble op that waits
on a DMA blocks everything emitted behind it on the same queue.  Layout:
  gpsimd: x group loads (back-to-back, no cross-engine waits) + squares
  vector: mask compares + XH casts + sq reduces
  scalar: Sign mask compares only (plus tail)
  sync:   staging writes + XT/RM DMA-transposes (write g precedes
          transpose g, which needs it anyway) + half the out DMAs
The j-side groups are software-pipelined into the main loop with a 2-group
lead.  The PE is additionally kept busy through the preamble with dummy
matmuls: TensorE downclocks 2.4->1.2 GHz after ~3.4us of idle (HAM), and
a cold start would halve matmul throughput for the first stretch.
"""

import sys

sys.path.insert(0, "/opt/trn_rl_repo")

import numpy as np

import concourse.bass as bass
import concourse.tile as tile
from concourse import bacc, masks, mybir
from concourse.bass_utils import run_bass_kernel_spmd

F32 = mybir.dt.float32
F16 = mybir.dt.float16
AF = mybir.ActivationFunctionType
OP = mybir.AluOpType

N = 8192
D = 64
NCORES = 8
ROWS = N // NCORES          # 1024 rows per core
JT = N // 128               # 64 j-tiles
IT = ROWS // 128            # 8 i-tiles
NG = 16                     # preamble column groups
TPG = JT // NG              # j-tiles per group
R2 = 11.0 * 11.0
K1 = (np.e - 1.0) / 2.0
LT = 3                      # C-pass lag in t units
GLEAD = 3                   # j-side rest-of-group emission lead (in groups)
LLEAD = 16                  # j-side cast-load emission lead (in groups)
NWARM = 10                  # PE warm-up dummy matmuls through the preamble


def _body(nc, tc, pools, xf, xi, outd, dram):
    const, scratch, gpool, mpool, apool, spool = pools
    ts = bass.ts

    # ---------------- persistent tiles ----------------
    XT = const.tile([128, N], F16, tag="XT")        # rows 0:64 x^T, row 64 ones
    # staging: row-major fp16 [x_j | 1 | pad], also the C-pass stationary.
    # Filled by fp32->fp16 cast-DMAs straight from DRAM (gpsimd SWDGE).
    XH = const.tile([128, JT * 128], F16, tag="XH")
    XH3 = XH[:].rearrange("p (t e) -> p t e", e=128)
    RM = const.tile([65, ROWS], F16, tag="RM")      # rows 0:64 x_i^T, row 64 -sq/2
    IDN2 = const.tile([128, 128], F32, tag="IDN2")
    biasA = const.tile([128, JT], F32, tag="biasA")
    thrD = const.tile([128, JT], F32, tag="thrD")
    IDN = const.tile([65, 65], F32, tag="IDN")
    IDN1 = const.tile([1, 1], F32, tag="IDN1")
    ONE1 = const.tile([128, 1], F16, tag="ONE1")
    DWT = const.tile([65, 512], F16, tag="DWT")     # warm-up dummy operand
    SAS = const.tile([65, 1], F32, tag="SAS")       # colsum of [x|1] over all j
    SP = apool.tile([1, 65 * TPG], F32, tag="SP")   # psum accumulator for SAS
    OUT2 = apool.tile([65, ROWS], F32, tag="OUT2")

    nc.vector.memset(DWT[:], 0.001)
    nc.vector.memset(ONE1[:], 1.0)
    nc.vector.memset(IDN1[:], 1.0)

    # PE warm-up: HAM needs sustained busy; no deps besides DWT
    for w in range(NWARM):
        dum = gpool.tile([128, 512], F32, tag="G")
        nc.tensor.matmul(dum[:], DWT[:, 0:128], DWT[:], start=True, stop=True)

    # gpsimd: ones column for every j-tile of XH, then the critical
    # cast-loads, BEFORE any other gpsimd work
    nc.gpsimd.memset(XH3[:, :, D : D + 1], 1.0)
    masks.make_identity(nc, IDN2[:])

    # i-side x load, fp32, partition = i-within-tile (row t*128 + p);
    # RM is built from 8 PE transposes during the otherwise-idle
    # preamble (no DRAM round-trip, no XBAR/SWDGE-ring involvement)
    xitp = scratch.tile([128, IT * 65], F32, tag="xitp")
    xitp3 = xitp[:].rearrange("p (t e) -> p t e", e=65)
    nc.gpsimd.dma_start(xitp3[:, :, 0:D],
                        xi.rearrange("(t p) d -> p t d", p=128))

    # Batched cast loads: SWDGE issue cost is ~0.9us per DMA instruction
    # regardless of size, so batch groups per load -- small batches first
    # (latency for the loop start), large ones later (throughput).
    # Within a batch of nt tiles, partition p holds nt consecutive rows
    # of the batch slab; j-tile labeling is rows {base + nt*p + local}.
    # The relabeling is invisible outside (j only ever summed over;
    # XT/XH/thr/SALL all derive from this load).  The fp32->fp16 cast
    # happens in the DMA; pad cols 65:128 stay garbage (only XT rows
    # 0:65 / XH cols 0:65 are ever read).
    BATCHES = [[0], [1], [2, 3], [4, 5, 6, 7], [8, 9, 10, 11],
               [12, 13, 14, 15]]

    def emit_load(b):
        bat = BATCHES[b]
        g0, g1 = bat[0], bat[-1] + 1
        nt = (g1 - g0) * TPG
        nc.gpsimd.dma_start(
            XH3[:, g0 * TPG : g1 * TPG, 0:D],
            xf[g0 * TPG * 128 : g1 * TPG * 128, :]
            .rearrange("(p t) d -> p t d", t=nt),
        )

    # all j-side cast-loads fire here, right after the xi load: the XBAR
    # transposes share the 8-deep SWDGE descriptor ring with every
    # gpsimd DMA, so the ring must hold exactly [xi, L0..L5, RM-T,
    # T0..T5] in landing order -- each transpose's ring-slot wait then
    # points at an early load.
    for _b in range(4):
        emit_load(_b)

    # ---------------- i-side rest -------------
    s2i = scratch.tile([128, IT * D], F32, tag="s2i")
    nc.vector.tensor_tensor(s2i[:].rearrange("p (t d) -> p t d", d=D),
                            xitp3[:, :, 0:D], xitp3[:, :, 0:D], OP.mult)
    sqit = scratch.tile([128, IT], F32, tag="sqit")
    nc.vector.tensor_reduce(sqit[:], s2i[:].rearrange("p (t d) -> p t d", d=D),
                            axis=mybir.AxisListType.X, op=OP.add)
    nc.vector.tensor_scalar(
        xitp3[:, :, D : D + 1].rearrange("p t u -> p (t u)"),
        sqit[:], -0.5, None, OP.mult)
    for tau in range(IT):
        rmps = gpool.tile([65, 128], F32, tag="G")
        nc.tensor.transpose(rmps[:], xitp3[:, tau, :], IDN2[:])
        nc.vector.tensor_copy(RM[0:65, ts(tau, 128)], rmps[:])

    # trigger the Sign act-table load early so it overlaps the preamble
    dumm = spool.tile([128, 1], F32, tag="dumm")
    nc.scalar.activation(dumm[:], ONE1[:], AF.Sign)

    # ---------------- j-side group chain (software-pipelined) ----------
    # one DRAM staging tile PER BATCH: a shared tile would add false
    # whole-tile write-after-read deps between batch k+1's writes and
    # batch k's transpose
    stages = {b: dram.tile([(bat[-1] + 1 - bat[0]) * TPG * 128, 128], F16,
                           name=f"stage{b}", tag=f"stage{b}")
              for b, bat in enumerate(BATCHES)}

    def emit_stage(b):
        # per-group staging writes (a multi-group write AP miscompiles)
        # alternating sync/scalar HWDGE queues (NOT gpsimd: that would
        # add SWDGE ring pressure ahead of the transposes); ONE batched
        # DMA transpose per batch (~1.2us fixed cost per instruction).
        bat = BATCHES[b]
        g0, g1 = bat[0], bat[-1] + 1
        stage = stages[b]
        for g in bat:
            rows = TPG * 128
            seg = stage[(g - g0) * rows : (g - g0 + 1) * rows, :]
            weng = nc.sync if g % 2 == 0 else nc.scalar
            weng.dma_start(seg.rearrange("(t p) e -> p t e", p=128),
                           XH[:, g * rows : (g + 1) * rows])
        nc.sync.dma_start(XT[:, g0 * 512 : g1 * 512], stage[:],
                          transpose=True)

    def emit_rest(g):
        # sq_j (from the quantized fp16, consistent with the matmul)
        xh3 = XH3[:, g * TPG : (g + 1) * TPG, :]
        s2 = scratch.tile([128, TPG * D], F32, tag="s2")
        nc.vector.tensor_tensor(s2[:].rearrange("p (t d) -> p t d", d=D),
                                xh3[:, :, 0:D], xh3[:, :, 0:D], OP.mult)
        sl = slice(g * TPG, (g + 1) * TPG)
        nc.vector.tensor_reduce(biasA[:, sl],
                                s2[:].rearrange("p (t d) -> p t d", d=D),
                                axis=mybir.AxisListType.X, op=OP.add)
        nc.vector.tensor_scalar(thrD[:, sl], biasA[:, sl], 0.5, -R2 / 2.0,
                                OP.mult, OP.add)
        nc.vector.tensor_scalar(biasA[:, sl], biasA[:, sl], -0.5, R2 / 2.0,
                                OP.mult, OP.add)


    def emit_sas_reduce():
        # SP over t -> [1, 65] (copy to SBUF, tree of adds), PE transpose
        spsb = spool.tile([1, 65 * TPG], F32, tag="spsb")
        nc.vector.tensor_copy(spsb[:], SP[:])
        sA = spool.tile([1, 65], F32, tag="sA")
        nc.vector.tensor_tensor(sA[:], spsb[:, 0:65], spsb[:, 65:130], OP.add)
        sB = spool.tile([1, 65], F32, tag="sB")
        nc.vector.tensor_tensor(sB[:], spsb[:, 130:195], spsb[:, 195:260], OP.add)
        srow = spool.tile([1, 65], F32, tag="srow")
        nc.vector.tensor_tensor(srow[:], sA[:], sB[:], OP.add)
        spt = gpool.tile([65, 1], F32, tag="G")
        nc.tensor.transpose(spt[:], srow[:], IDN1[:])
        nc.vector.tensor_copy(SAS[:], spt[:])
        b1sb = spool.tile([65, 1], F32, tag="b1sb")
        nc.vector.tensor_scalar(b1sb[:], SAS[:], 1.0 + K1, None, OP.mult)
        return b1sb

    for b in range(3):
        emit_stage(b)                    # groups 0-3
    for g in range(GLEAD):
        emit_rest(g)

    # ------- main loop over j-tiles, pass C lagged by LT tiles ----
    mks = {}
    b1sb = None
    for t in range(JT + LT):
        if t == TPG:
            emit_stage(3)               # groups 4-7
        elif t == 2 * TPG:
            emit_load(4)
        elif t == 4 * TPG:
            emit_load(5)
        elif t == 5 * TPG:
            emit_stage(4)               # groups 8-11
        elif t == 9 * TPG:
            emit_stage(5)               # groups 12-15
        if t % TPG == 0 and t // TPG + GLEAD < NG:
            emit_rest(t // TPG + GLEAD)
        if t == 40:
            masks.make_identity(nc, IDN[:])
            # colsum: SP[0, (t e)] += sum_p [x|1], per group; deferred to
            # here so the XH cast-loads have all landed (no PE waits)
            for g in range(NG):
                nc.tensor.matmul(SP[:], ONE1[:],
                                 XH3[:, g * TPG : (g + 1) * TPG, 0:65],
                                 start=(g == 0), stop=(g == NG - 1))
        if t == TPG * (NG - GLEAD) + 1:
            b1sb = emit_sas_reduce()   # right after the last SALL matmul
        if t < JT:
            W = XT[0:65, ts(t, 128)]
            Gh0 = gpool.tile([128, 512], F32, tag="G")
            nc.tensor.matmul(Gh0[:], W, RM[0:65, 0:512], start=True, stop=True)
            Gh1 = gpool.tile([128, 512], F32, tag="G")
            nc.tensor.matmul(Gh1[:], W, RM[0:65, 512:1024], start=True, stop=True)
            mk0 = mpool.tile([128, 512], F16, tag="mk")
            nc.vector.tensor_scalar(mk0[:], Gh0[:], thrD[:, t : t + 1],
                                    2.0, OP.is_ge, OP.mult)
            mk1 = mpool.tile([128, 512], F16, tag="mk")
            nc.scalar.activation(mk1[:], Gh1[:], AF.Sign,
                                 bias=biasA[:, t : t + 1])
            mks[t] = (mk0, mk1)
        if t >= LT:
            u = t - LT
            xws = XH[:, 128 * u : 128 * u + 65]
            m0, m1 = mks.pop(u)
            nc.tensor.matmul(OUT2[:, 0:512], xws, m0[:],
                             start=(u == 0), stop=(u == JT - 1))
            nc.tensor.matmul(OUT2[:, 512:1024], xws, m1[:],
                             start=(u == 0), stop=(u == JT - 1))

    # ---------------- tail (per i-chunk, DVE/ACT alternating) -----------
    # emit all the scale stages first so the PE transposes and the DVE
    # recip/mult stages pipeline across chunks
    pcs = []
    for c in range(IT):
        bap = SAS if c < IT // 2 else b1sb
        pc = spool.tile([65, 128], F32, tag=f"pc{c}")
        if c % 2 == 0:
            nc.vector.tensor_scalar(pc[:], OUT2[:, ts(c, 128)], K1, bap[:],
                                    OP.mult, OP.add)
        else:
            nc.scalar.activation(pc[:], OUT2[:, ts(c, 128)], AF.Identity,
                                 bias=bap[:], scale=K1)
        pcs.append(pc)
    for c in range(IT):
        pt = gpool.tile([128, 65], F32, tag="G")
        nc.tensor.transpose(pt[:], pcs[c][:], IDN[:])
        dinv = spool.tile([128, 1], F32, tag="dinv")
        nc.vector.reciprocal(dinv[:], pt[:, D : D + 1])
        ot = spool.tile([128, D], F32, tag="ot")
        nc.vector.tensor_scalar(ot[:], pt[:, 0:D], dinv[:], None, OP.mult)
        eng = nc.sync if c % 2 == 0 else nc.scalar
        eng.dma_start(outd[ts(c, 128), :], ot[:])


def build_module(loop_n=1, scope='full'):
    nc = bacc.Bacc("TRN2", target_bir_lowering=False, debug=False,
                   num_devices=NCORES)
    xf_d = nc.dram_tensor("xf", [N, D], F32, kind="ExternalInput")
    xi_d = nc.dram_tensor("xi", [ROWS, D], F32, kind="ExternalInput")
    out_d = nc.dram_tensor("out", [ROWS, D], F32, kind="ExternalOutput")

    with tile.TileContext(nc) as tc:
        with (
            tc.tile_pool(name="const", bufs=1) as const,
            tc.tile_pool(name="scratch", bufs=5) as scratch,
            tc.tile_pool(name="gpool", bufs=5, space="PSUM") as gpool,
            tc.tile_pool(name="acc", bufs=1, space="PSUM") as apool,
            tc.tile_pool(name="mk", bufs=12) as mpool,
            tc.tile_pool(name="small", bufs=3) as spool,
            tc.tile_pool(name="dram", bufs=8, space="DRAM") as dram,
        ):
            pools = (const, scratch, gpool, mpool, apool, spool)
            args = (nc, tc, pools, xf_d.ap(), xi_d.ap(), out_d.ap(), dram)
            if loop_n == 1:
                _body(*args)
            else:
                with tc.For_i(0, loop_n) as _:
                    _body(*args)
    nc.finalize()
    return nc


_module_cache = {}


def _get_module(loop_n=1):
    if loop_n not in _module_cache:
        _module_cache[loop_n] = build_module(loop_n)
    return _module_cache[loop_n]


def kernel(x, adj=None):
    x = np.ascontiguousarray(np.asarray(x, dtype=np.float32))
    assert x.shape == (N, D)
    nc = _get_module(1)
    in_maps = [
        {"xf": x, "xi": x[c * ROWS : (c + 1) * ROWS]} for c in range(NCORES)
    ]
    res = run_bass_kernel_spmd(nc, in_maps, core_ids=list(range(NCORES)))
    return np.concatenate([res.results[c]["out"] for c in range(NCORES)], axis=0)
